# revision 39
# baseline (speedup 1.0000x reference)
"""Trainium2 Bass kernel for nn_Attention (B=4, N=2048, C=1024, H=16, D=64).

Sharding: 8 cores; core c handles batch b=c//2 and heads [8*(c%2), 8*(c%2)+8).
Each core computes qkv projection for its 512 channels, RMSNorm(q/k),
attention over its 8 heads, and a partial output projection (contraction over
its 512 channels). Host sums the two partial proj outputs per batch.

Schedule: the ACT-engine exp stream (256 x [128,1024] activations, ~290us)
is the critical path. All non-score PE work (qkv, V, stats, proj, norm) is
chopped into ~0.5-1.7us "fill quanta" drained by a leaky-bucket budget
between score pairs, so the in-order PE queue never starves the exp stream.
attn@V matmuls lag the sc/exp stream by >=2 j-tiles (po queue) so the PE
never head-of-line blocks on the exp it just issued.

PSUM: sc pool 2x[128,1024] (4 banks) + fill pool 2x[128,<=512] (2 banks)
+ po pool 2x[65,512] (2 banks) = 8 banks.

Stats: per head-pair, variances for (q,k)x(2 heads)x2048 tokens pack into
two [128,512] PSUM tiles (8 live rows each, 32-aligned for tile_position),
so Ln+Exp cost ~2.4us/hp on ACT and fit 1-bank fill tiles. ln/exp share one
pinned activation table (no table swaps).
"""

import os
import numpy as np
import ml_dtypes

B, N, C, H, D = 4, 2048, 1024, 16, 64
NCORES = 8
HPC = 8           # heads per core
CH = HPC * D      # 512 channels per core
VSEG = 2 * D + 4  # 132 cols per pair in v_aug: [64 v | 1 | 1][64 v | 1 | 1]
VW = 4 * VSEG     # 528
EPS = 1e-6

_CACHE = {}
LAST_RESULT = [None]


def _build_nc():
    import concourse.tile as tile
    import concourse.mybir as mybir
    from concourse import bacc

    F32 = mybir.dt.float32
    F32R = mybir.dt.float32r
    BF16 = mybir.dt.bfloat16
    AF = mybir.ActivationFunctionType

    class PinnedBacc(bacc.Bacc):
        """Route ln/exp to the shared natural_log_exp table so the ACT
        engine never swaps activation tables mid-kernel (each swap is
        ~1.3us and stalls the exp stream)."""

        def insert_act_table_loads(self):
            import bass_rust as _bass_rust
            from concourse.hw_specs import get_activation_tables
            has_activation = any(
                isinstance(i, mybir.InstActivation)
                for b in self.main_func.blocks
                for i in b.instructions
            )
            if not has_activation:
                return
            tables = []
            for name, fns in get_activation_tables(self.m.arch).items():
                if name != "natural_log_exp_and_others":
                    fns = {f for f in fns
                           if f.name.lower() not in ("exp", "ln")}
                tables.append((name, fns))
            _bass_rust.insert_act_table_loads(self, tables)

    nc = PinnedBacc("TRN2", target_bir_lowering=False, debug=False,
                    num_devices=NCORES)

    XTB = nc.dram_tensor("XTB", [C, N], BF16, kind="ExternalInput")
    WQ = nc.dram_tensor("WQ", [C, CH], BF16, kind="ExternalInput")
    WK = nc.dram_tensor("WK", [C, CH], BF16, kind="ExternalInput")
    WVA = nc.dram_tensor("WVA", [C, VW], BF16, kind="ExternalInput")
    WP = nc.dram_tensor("WP", [CH, C], BF16, kind="ExternalInput")
    BQK = nc.dram_tensor("BQK", [128, 8], F32, kind="ExternalInput")
    BVA = nc.dram_tensor("BVA", [128, VW], F32, kind="ExternalInput")
    BP = nc.dram_tensor("BP", [128, C], F32, kind="ExternalInput")
    QKN = nc.dram_tensor("QKN", [128, 2], F32, kind="ExternalInput")
    BLK2 = nc.dram_tensor("BLK2", [128, 64], BF16, kind="ExternalInput")
    SEL4 = nc.dram_tensor("SEL4", [128, 512], BF16, kind="ExternalInput")
    EPSV = nc.dram_tensor("EPSV", [128, 1], F32, kind="ExternalInput")
    ONESB = nc.dram_tensor("ONESB", [97, 64], F32R, kind="ExternalInput")
    Y = nc.dram_tensor("Y", [N, C], F32, kind="ExternalOutput")

    NT = N // 128          # 16 token tiles
    KT = C // 128          # 8 contraction tiles
    NCHUNK = N // 512      # 4 i-chunks of 512 tokens

    with tile.TileContext(nc) as tc:
        from contextlib import ExitStack
        from collections import deque
        with ExitStack() as ctx:
            const_p = ctx.enter_context(tc.tile_pool(name="const", bufs=1))
            xtb_p = ctx.enter_context(tc.tile_pool(name="xtb", bufs=8))
            outT_p = ctx.enter_context(tc.tile_pool(name="outT", bufs=4))
            wp_p = ctx.enter_context(tc.tile_pool(name="wp", bufs=4))
            y_p = ctx.enter_context(tc.tile_pool(name="y", bufs=2))

            sc_ps = ctx.enter_context(
                tc.tile_pool(name="scps", bufs=2, space="PSUM"))
            fill_ps = ctx.enter_context(
                tc.tile_pool(name="fillps", bufs=2, space="PSUM"))
            po_ps = ctx.enter_context(
                tc.tile_pool(name="po", bufs=2, space="PSUM"))

            # small constants
            bqk_sb = const_p.tile([128, 8], F32, tag="bqk")
            nc.sync.dma_start(bqk_sb[:], BQK.ap()[:, :])
            qkn_sb = const_p.tile([128, 2], F32, tag="qkn")
            nc.sync.dma_start(qkn_sb[:], QKN.ap()[:, :])
            blk2_sb = const_p.tile([128, 64], BF16, tag="blk2")
            nc.sync.dma_start(blk2_sb[:], BLK2.ap()[:, :])
            sel4_sb = const_p.tile([128, 512], BF16, tag="sel4")
            nc.sync.dma_start(sel4_sb[:], SEL4.ap()[:, :])
            epsv_sb = const_p.tile([128, 1], F32, tag="epsv")
            nc.sync.dma_start(epsv_sb[:], EPSV.ap()[:, :])
            onesb_sb = const_p.tile([97, 64], F32R, tag="onesb")
            nc.sync.dma_start(onesb_sb[:], ONESB.ap()[:, :])

            xtb_tiles = []
            for kt in range(KT):
                t = xtb_p.tile([128, N], BF16, tag="xtb", name=f"xtb{kt}")
                for h in range(2):
                    nc.sync.dma_start(
                        t[:, h * 1024:(h + 1) * 1024],
                        XTB.ap()[kt * 128:(kt + 1) * 128,
                                 h * 1024:(h + 1) * 1024])
                xtb_tiles.append(t)

            # ---------------- V tiles (bf16) -------------------------------
            v_ctx = tc.tile_pool(name="v", bufs=16)
            v_p = v_ctx.__enter__()
            v_tiles = []
            for nt in range(NT):
                v_tiles.append(
                    v_p.tile([128, VW], BF16, tag="v", name=f"vt{nt}"))

            pair_ctx = ExitStack()
            w_p = pair_ctx.enter_context(tc.tile_pool(name="w", bufs=1))
            qtb_p = pair_ctx.enter_context(tc.tile_pool(name="qtb", bufs=1))
            sq_p = pair_ctx.enter_context(tc.tile_pool(name="sq", bufs=4))
            qtn_p = pair_ctx.enter_context(tc.tile_pool(name="qtn", bufs=2))
            stat_p = pair_ctx.enter_context(tc.tile_pool(name="stat", bufs=4))
            rcp_p = pair_ctx.enter_context(tc.tile_pool(name="rcp", bufs=1))
            ex_p = pair_ctx.enter_context(tc.tile_pool(name="ex", bufs=6))

            wv_ctx = tc.tile_pool(name="wv", bufs=1)
            wv_p = wv_ctx.__enter__()

            state = {}

            # ============ fill-quantum / po scheduling machinery ==========
            fillq = deque()       # (cost_ns, emit_fn)
            poq = deque()         # (hp, ic, jt, emit_fn)
            sched = {"v_done": 0, "jt_global": 0, "budget": 0.0,
                     "qtn_q": [0] * 4, "ktn_q": [0] * 4}
            # fill ns per emitted jt (ACT slack); hp0 is PE-bound (V),
            # hp1 drains hp0's po backlog, hp3 is proj-heavy
            JT_BUDGET = [750.0, 620.0, 560.0, 600.0]

            def push_fill(cost, fn):
                fillq.append((cost, fn))

            def pop_fill_budget():
                while fillq and fillq[0][0] <= sched["budget"]:
                    cost, fn = fillq.popleft()
                    sched["budget"] -= cost
                    fn()

            def flush_fill(n=None):
                cnt = 0
                while fillq and (n is None or cnt < n):
                    cost, fn = fillq.popleft()
                    fn()
                    cnt += 1
                sched["budget"] = min(sched["budget"], 0.0)

            # ---------------- emit helpers --------------------------------
            def emit_v_nt(nt, vh):
                # one quantum: half of v-tile nt (8 matmuls of 264 cols)
                cs = vh * (VW // 2)
                ps = fill_ps.tile([128, VW // 2], F32, tag="fill")
                for kt in range(KT):
                    nc.tensor.matmul(
                        ps[:],
                        xtb_tiles[kt][:, nt * 128:(nt + 1) * 128],
                        wv_sb[:, kt * VW + cs:kt * VW + cs + VW // 2],
                        start=(kt == 0), stop=(kt == KT - 1))
                nc.vector.tensor_add(
                    v_tiles[nt][:, cs:cs + VW // 2], ps[:],
                    bva_sb[:, cs:cs + VW // 2])
                if vh == 1:
                    sched["v_done"] = nt + 1

            def emit_w_loads(hp):
                wq_sb = w_p.tile([128, C], BF16, tag="wq")
                wk_sb = w_p.tile([128, C], BF16, tag="wk")
                for kt in range(KT):
                    nc.sync.dma_start(
                        wq_sb[:, kt * 128:(kt + 1) * 128],
                        WQ.ap()[kt * 128:(kt + 1) * 128,
                                hp * 128:(hp + 1) * 128])
                    nc.sync.dma_start(
                        wk_sb[:, kt * 128:(kt + 1) * 128],
                        WK.ap()[kt * 128:(kt + 1) * 128,
                                hp * 128:(hp + 1) * 128])
                st = state.setdefault(hp, {})
                st["wq"], st["wk"] = wq_sb, wk_sb
                st["qT_b"] = qtb_p.tile([128, N], BF16, tag="qtb",
                                        name=f"qTb{hp}")
                st["kT_b"] = qtb_p.tile([128, N], BF16, tag="ktb",
                                        name=f"kTb{hp}")

            def emit_qkv_pass(hp, is_k, qq):
                # one quantum: 8-ktile accumulation into [128,512] + bias
                st = state[hp]
                wsb = st["wk"] if is_k else st["wq"]
                dst = st["kT_b"] if is_k else st["qT_b"]
                bcol = (4 + hp) if is_k else hp
                ssl = slice(qq * 512, (qq + 1) * 512)
                ps = fill_ps.tile([128, 512], F32, tag="fill")
                for kt in range(KT):
                    nc.tensor.matmul(
                        ps[:], wsb[:, kt * 128:(kt + 1) * 128],
                        xtb_tiles[kt][:, ssl],
                        start=(kt == 0), stop=(kt == KT - 1))
                nc.vector.tensor_scalar(
                    dst[:, ssl], ps[:], bqk_sb[:, bcol:bcol + 1],
                    None, op0=mybir.AluOpType.add)

            def emit_stats_var_T(hp, T):
                # one quantum: variance tile T (token quarters 2T, 2T+1).
                # rows: q@64s..+1, k@64s+32..+33 for s in {0,1}; one Ln +
                # one Exp of [128,512] each.
                st = state[hp]
                qT_b, kT_b = st["qT_b"], st["kT_b"]
                vps = fill_ps.tile([128, 512], F32, tag="fill",
                                   name=f"vps{hp}_{T}")
                for s in range(2):
                    qq = 2 * T + s
                    ssl = slice(qq * 512, (qq + 1) * 512)
                    for qk, src_t in ((0, qT_b), (1, kT_b)):
                        sq = sq_p.tile([128, 512], BF16, tag="sq")
                        nc.gpsimd.tensor_mul(sq[:], src_t[:, ssl],
                                             src_t[:, ssl])
                        r = 64 * s + 32 * qk
                        bcols = slice(32, 64) if qk else slice(0, 32)
                        nc.tensor.matmul(
                            vps[r:r + 32, :], blk2_sb[:, bcols], sq[:],
                            start=True, stop=True, tile_position=(0, r))
                lg = stat_p.tile([128, 512], F32, tag="lg",
                                 name=f"lg{hp}_{T}")
                nc.scalar.activation(lg[:], vps[:], AF.Ln, bias=epsv_sb[:])
                rs = stat_p.tile([128, 512], BF16, tag="rs",
                                 name=f"rs{hp}_{T}")
                nc.scalar.activation(rs[:], lg[:], AF.Exp, scale=-0.5)
                st.setdefault("rs", {})[T] = rs

            def emit_stats_bcast_q(hp, is_k, qq):
                # one quantum: rsqrt broadcast + normalize for one
                # (q/k, token-quarter): [128,512]
                st = state[hp]
                src_t = st["kT_b"] if is_k else st["qT_b"]
                if "qTn" not in st:
                    st["qTn"] = qtn_p.tile([128, N], BF16, tag="qtn",
                                           name=f"qTn{hp}")
                    st["kTn"] = qtn_p.tile([128, N], BF16, tag="ktn",
                                           name=f"kTn{hp}")
                dstn = st["kTn"] if is_k else st["qTn"]
                rs = st["rs"][qq // 2]
                vi = 2 * (qq % 2) + int(is_k)
                ssl = slice(qq * 512, (qq + 1) * 512)
                bc = fill_ps.tile([128, 512], F32, tag="fill", name="bc")
                nc.tensor.matmul(
                    bc[:], sel4_sb[:, vi * 128:(vi + 1) * 128], rs[:],
                    start=True, stop=True)
                wcol = 1 if is_k else 0
                nc.vector.scalar_tensor_tensor(
                    dstn[:, ssl], src_t[:, ssl],
                    qkn_sb[:, wcol:wcol + 1], bc[:],
                    op0=mybir.AluOpType.mult,
                    op1=mybir.AluOpType.mult)
                key = "ktn_q" if is_k else "qtn_q"
                sched[key][hp] = max(sched[key][hp], qq + 1)

            outT_tiles = []

            def emit_attention_start(hp):
                st = state[hp]
                outT = outT_p.tile([128, N], BF16, tag="outT",
                                   name=f"outT{hp}")
                outT_tiles.append(outT)
                st["outT"] = outT
                # hp<3: dense [8,512] pack (rows 2*ic+hh) -> one batched
                # reciprocal. hp3: rows 32*ic+hh so per-ic reciprocal
                # reads start 32-aligned (engine-op requirement).
                st["den"] = rcp_p.tile([98, 512] if hp == 3 else [8, 512],
                                       BF16, tag="den_pack",
                                       bufs=2, name=f"den{hp}")
                st["po_sbs"] = {}

            def emit_po_jt(hp, ic, jt, poA, poB):
                st = state[hp]
                ex = st["exs"].pop((ic, jt))
                vbase = hp * VSEG
                nc.tensor.matmul(
                    poA[:], v_tiles[jt][:, vbase:vbase + 65],
                    ex[:, 0:512], start=(jt == 0), stop=(jt == NT - 1))
                nc.tensor.matmul(
                    poB[:],
                    v_tiles[jt][:, vbase + VSEG // 2:
                                vbase + VSEG // 2 + 65],
                    ex[:, 512:1024], start=(jt == 0), stop=(jt == NT - 1))
                if jt == NT - 1:
                    emit_den_extract(hp, ic, poA, poB)

            def emit_den_extract(hp, ic, poA, poB):
                st = state[hp]
                for hh, (po, rowoff) in enumerate(((poA, 0), (poB, 64))):
                    idx = ic * 2 + hh
                    po_sb = rcp_p.tile([65, 512], BF16, tag="po_sb",
                                       name=f"po_sb{hp}_{idx}", bufs=8)
                    nc.vector.tensor_copy(po_sb[:], po[:, :])
                    drow = (32 * ic + hh) if hp == 3 else (2 * ic + hh)
                    nc.sync.dma_start(
                        st["den"][drow:drow + 1, :], po_sb[64:65, :])
                    st["po_sbs"][idx] = (po_sb, rowoff, ic)
                if hp == 3:
                    push_hp3_post(ic)
                elif ic == 3:
                    push_norm_quanta(hp)

            def _rcp_al_dma(hp, ic, rcp_pack, row0):
                st = state[hp]
                rcp_al = rcp_p.tile([33, 512], F32, tag="rcp_al",
                                    bufs=4, name=f"rcpa{hp}_{ic}")
                st[f"rcp_al{ic}"] = rcp_al
                for i in range(2):
                    nc.sync.dma_start(rcp_al[32 * i:32 * i + 1, :],
                                      rcp_pack[row0 + i:row0 + i + 1, :])

            def emit_norm_rcp(hp):
                st = state[hp]
                rcp_pack = rcp_p.tile([8, 512], F32, tag="rcp_pack",
                                      bufs=2, name=f"rcpp{hp}")
                with nc.allow_low_precision(
                        reason="f32 reciprocal of softmax denominators"):
                    nc.vector.reciprocal(rcp_pack[:], st["den"][:, :])
                for ic in range(NCHUNK):
                    _rcp_al_dma(hp, ic, rcp_pack, 2 * ic)

            def emit_norm_rcp_ic(hp, ic):
                st = state[hp]
                rcp_pack = rcp_p.tile([2, 512], F32, tag="rcp_pack",
                                      bufs=2, name=f"rcpp{hp}_{ic}")
                with nc.allow_low_precision(
                        reason="f32 reciprocal of softmax denominators"):
                    nc.vector.reciprocal(
                        rcp_pack[:], st["den"][32 * ic:32 * ic + 2, :])
                _rcp_al_dma(hp, ic, rcp_pack, 0)

            def emit_norm_apply(hp, ic):
                # broadcast rcp rows via ONES matmul and scale po -> outT
                st = state[hp]
                outT = st["outT"]
                isl = slice(ic * 512, (ic + 1) * 512)
                rcp_al = st[f"rcp_al{ic}"]
                for hh in range(2):
                    po_sb, rowoff, _ = st["po_sbs"][2 * ic + hh]
                    r = 32 * hh
                    rb_ps = fill_ps.tile([64, 512], F32, tag="fill",
                                         name="rb_ps")
                    nc.tensor.matmul(
                        rb_ps[:], onesb_sb[r:r + 1, :],
                        rcp_al[r:r + 1, :].bitcast(F32R),
                        start=True, stop=True, tile_position=(r, 0))
                    nc.vector.tensor_mul(
                        outT[rowoff:rowoff + 64, isl], po_sb[0:64, :],
                        rb_ps[:])

            y_sbs = {}

            def emit_proj_sub(nt, sub):
                # one quantum: half of proj output tile nt
                ps = fill_ps.tile([128, 512], F32, tag="fill")
                for kt in range(4):
                    nc.tensor.matmul(
                        ps[:], outT_tiles[kt][:, nt * 128:(nt + 1) * 128],
                        wp_tiles[kt][:, sub * 512:(sub + 1) * 512],
                        start=(kt == 0), stop=(kt == 3))
                if sub == 0:
                    y_sbs[nt] = y_p.tile([128, C], F32, tag="y",
                                         name=f"y{nt}")
                y_sb = y_sbs[nt]
                ssl = slice(sub * 512, (sub + 1) * 512)
                nc.vector.tensor_add(y_sb[:, ssl], ps[:], bp_sb[:, ssl])
                if sub == 1:
                    nc.sync.dma_start(Y.ap()[nt * 128:(nt + 1) * 128, :],
                                      y_sb[:])

            def push_norm_quanta(hp):
                # after hp's last den row lands: batched rcp, then one
                # apply-quantum per i-chunk
                push_fill(900, lambda hp=hp: emit_norm_rcp(hp))
                for ic in range(NCHUNK):
                    push_fill(500, lambda hp=hp, ic=ic:
                              emit_norm_apply(hp, ic))

            def push_hp3_post(ic):
                # hp3 per-ic: rcp + apply + the 4 proj tiles it unlocks
                push_fill(600, lambda ic=ic: emit_norm_rcp_ic(3, ic))
                push_fill(500, lambda ic=ic: emit_norm_apply(3, ic))
                for nt in range(4 * ic, 4 * ic + 4):
                    for sub in range(2):
                        push_fill(900, lambda nt=nt, sub=sub:
                                  emit_proj_sub(nt, sub))

            # ---------------- attention jt stream --------------------------
            def emit_sc_jt(hp, ic, jt):
                # correctness guard: kTn quarter jt//4 and qTn quarter ic
                # must be emitted before this score pair reads them
                while (sched["ktn_q"][hp] <= jt // 4
                       or sched["qtn_q"][hp] <= ic):
                    assert fillq, "sc guard: missing bcast quanta"
                    flush_fill(n=1)
                st = state[hp]
                qTn, kTn = st["qTn"], st["kTn"]
                isl = slice(ic * 512, (ic + 1) * 512)
                jsl = slice(jt * 128, (jt + 1) * 128)
                sc = sc_ps.tile([128, 1024], F32, tag="sc")
                nc.tensor.matmul(
                    sc[:, 0:512], kTn[0:64, jsl], qTn[0:64, isl],
                    start=True, stop=True, tile_position=(0, 0))
                nc.tensor.matmul(
                    sc[:, 512:1024], kTn[64:128, jsl], qTn[64:128, isl],
                    start=True, stop=True, tile_position=(64, 0))
                ex = ex_p.tile([128, 1024], BF16, tag="ex")
                nc.scalar.activation(ex[:], sc[:], AF.Exp)
                st.setdefault("exs", {})[(ic, jt)] = ex

            def pop_po(max_n):
                n = 0
                while poq and n < max_n:
                    hp, ic, jt, born = poq[0]
                    if sched["v_done"] <= jt:
                        break
                    if sched["jt_global"] - born < 2:
                        break
                    poq.popleft()
                    po_pair = po_tiles[(hp, ic)]
                    emit_po_jt(hp, ic, jt, po_pair[0], po_pair[1])
                    n += 1

            po_tiles = {}

            # ================= emission =====================================
            # DMA priority: wq/wk hp0 (gates first qkv), wv, then the
            # late-needed BVA/BP/WP constants.
            emit_w_loads(0)
            wv_sb = wv_p.tile([128, KT * VW], BF16, tag="wv")
            for kt in range(KT):
                nc.sync.dma_start(wv_sb[:, kt * VW:(kt + 1) * VW],
                                  WVA.ap()[kt * 128:(kt + 1) * 128, :])
            bva_sb = const_p.tile([128, VW], F32, tag="bva")
            nc.sync.dma_start(bva_sb[:], BVA.ap()[:, :])
            bp_sb = const_p.tile([128, C], F32, tag="bp")
            nc.sync.dma_start(bp_sb[:], BP.ap()[:, :])
            wp_tiles = []
            for kt in range(4):
                t = wp_p.tile([128, C], BF16, tag="wp")
                nc.sync.dma_start(t[:], WP.ap()[kt * 128:(kt + 1) * 128, :])
                wp_tiles.append(t)

            # pre-loop: only token quarters 0-1 of qkv(0) + stats T0 +
            # their bcasts run directly -- enough to start the first 8
            # j-tiles of scores. Quarters 2-3 drain as fill quanta under
            # the sc-guard (shortens the first-exp critical path ~9us).
            for qq in range(2):
                emit_qkv_pass(0, False, qq)
                emit_qkv_pass(0, True, qq)
            emit_stats_var_T(0, 0)
            for qq in range(2):
                emit_stats_bcast_q(0, False, qq)
                emit_stats_bcast_q(0, True, qq)
            for qq in range(2, 4):
                push_fill(1700, lambda qq=qq: emit_qkv_pass(0, False, qq))
                push_fill(1700, lambda qq=qq: emit_qkv_pass(0, True, qq))
            push_fill(900, lambda: emit_stats_var_T(0, 1))
            for qq in range(2, 4):
                for is_k in (False, True):
                    push_fill(250, lambda is_k=is_k, qq=qq:
                              emit_stats_bcast_q(0, is_k, qq))

            # hp0 fill: V (16 tiles, 2 quanta each) interleaved with
            # qkv(1); then stats(1) + bcast(1)
            emit_w_loads(1)
            qkv1 = [(False, qq) for qq in range(4)] + \
                   [(True, qq) for qq in range(4)]
            for nt in range(NT):
                push_fill(880, lambda nt=nt: emit_v_nt(nt, 0))
                push_fill(880, lambda nt=nt: emit_v_nt(nt, 1))
                if nt % 2 == 1 and qkv1:
                    is_k, qq = qkv1.pop(0)
                    push_fill(1700, lambda is_k=is_k, qq=qq:
                              emit_qkv_pass(1, is_k, qq))
            for T in range(2):
                push_fill(900, lambda T=T: emit_stats_var_T(1, T))
            for qq in range(4):
                for is_k in (False, True):
                    push_fill(250, lambda is_k=is_k, qq=qq:
                              emit_stats_bcast_q(1, is_k, qq))

            for hp in range(4):
                if hp >= 1:
                    # fill with a deadline this hp (qTn/kTn broadcast)
                    # must be fully emitted before this hp's first scores
                    flush_fill()
                emit_attention_start(hp)
                if hp == 1:
                    wv_ctx.__exit__(None, None, None)
                if 1 <= hp <= 2:
                    nhp = hp + 1
                    emit_w_loads(nhp)
                    for is_k in (False, True):
                        for qq in range(4):
                            push_fill(1700, lambda n=nhp, is_k=is_k, qq=qq:
                                      emit_qkv_pass(n, is_k, qq))
                    for T in range(2):
                        push_fill(900, lambda n=nhp, T=T:
                                  emit_stats_var_T(n, T))
                    for qq in range(4):
                        for is_k in (False, True):
                            push_fill(250, lambda n=nhp, is_k=is_k, qq=qq:
                                      emit_stats_bcast_q(n, is_k, qq))
                for ic in range(NCHUNK):
                    po_tiles[(hp, ic)] = (
                        po_ps.tile([65, 512], F32, tag="po",
                                   name=f"poA{hp}_{ic}"),
                        po_ps.tile([65, 512], F32, tag="po",
                                   name=f"poB{hp}_{ic}"))
                    for jt in range(NT):
                        emit_sc_jt(hp, ic, jt)
                        poq.append((hp, ic, jt, sched["jt_global"]))
                        sched["jt_global"] += 1
                        jb = JT_BUDGET[hp]
                        sched["budget"] = min(
                            sched["budget"] + jb, 4 * jb)
                        npo = 2
                        if len(poq) > 12:
                            npo = 4
                        elif len(poq) > 8:
                            npo = 3
                        pop_po(npo)
                        pop_fill_budget()

            # tail: drain remaining po + fill
            while poq:
                hp, ic, jt, _ = poq.popleft()
                po_pair = po_tiles[(hp, ic)]
                emit_po_jt(hp, ic, jt, po_pair[0], po_pair[1])
                flush_fill(n=1)
            flush_fill()

            pair_ctx.close()
            v_ctx.__exit__(None, None, None)

    nc.compile()
    return nc


def _core_inputs(c, x, W_qkv, b_qkv, W_proj, b_proj, qn_w, kn_w):
    b, half = c // 2, c % 2
    hbase = HPC * half
    co = hbase * D                      # channel offset of this core's heads

    xT = np.ascontiguousarray(x[b].T, dtype=np.float32)
    WQc = W_qkv[:, co:co + CH].astype(ml_dtypes.bfloat16)
    WKc = W_qkv[:, C + co:C + co + CH].astype(ml_dtypes.bfloat16)
    WVc = W_qkv[:, 2 * C + co:2 * C + co + CH]
    WVA = np.zeros((C, VW), dtype=np.float32)
    BVA1 = np.zeros((VW,), dtype=np.float32)
    bv = b_qkv[2 * C + co:2 * C + co + CH]
    for hp in range(4):
        for hh in range(2):
            s = hp * VSEG + hh * (VSEG // 2)
            WVA[:, s:s + D] = WVc[:, (2 * hp + hh) * D:(2 * hp + hh + 1) * D]
            BVA1[s:s + D] = bv[(2 * hp + hh) * D:(2 * hp + hh + 1) * D]
            BVA1[s + D] = 1.0  # ones column for softmax denominators
    WVA = WVA.astype(ml_dtypes.bfloat16)
    BVA = np.broadcast_to(BVA1, (128, VW)).copy()

    BQK = np.zeros((128, 8), dtype=np.float32)
    for hp in range(4):
        BQK[:, hp] = b_qkv[co + hp * 128:co + (hp + 1) * 128]
        BQK[:, 4 + hp] = b_qkv[C + co + hp * 128:C + co + (hp + 1) * 128]

    WPc = W_proj[co:co + CH, :].astype(ml_dtypes.bfloat16)
    BP = (np.broadcast_to(b_proj, (128, C)).copy() if half == 0
          else np.zeros((128, C), dtype=np.float32))
    QKN = np.stack([np.tile(qn_w, 2), np.tile(kn_w, 2)],
                   axis=1).astype(np.float32)

    # variance matmul weights: col 0/1 q h0/h1 (1/D), col 32/33 k h0/h1
    # (1.0; the 1/D and the 1/sqrt(D) score scale fold into the rsqrt)
    BLK2 = np.zeros((128, 64), dtype=np.float32)
    BLK2[0:64, 0] = 1.0 / D
    BLK2[64:128, 1] = 1.0 / D
    BLK2[0:64, 32] = 1.0
    BLK2[64:128, 33] = 1.0
    BLK2 = BLK2.astype(ml_dtypes.bfloat16)

    # rsqrt broadcast selectors: variant vi reads rs rows 32*vi (h0) and
    # 32*vi+1 (h1) onto channels 0-63 / 64-127
    SEL4 = np.zeros((128, 512), dtype=np.float32)
    for vi in range(4):
        SEL4[32 * vi, vi * 128:vi * 128 + 64] = 1.0
        SEL4[32 * vi + 1, vi * 128 + 64:vi * 128 + 128] = 1.0
    SEL4 = SEL4.astype(ml_dtypes.bfloat16)

    # packed variance tiles: rows 0-31 q (bias EPS), 32-63 k (bias EPS*D),
    # 64-95 q, 96-127 k
    EPSV = np.full((128, 1), EPS, dtype=np.float32)
    EPSV[32:64] = EPS * D
    EPSV[96:128] = EPS * D

    ONESB = np.zeros((97, 64), dtype=np.float32)
    for r in (0, 32, 64, 96):
        ONESB[r, :] = 1.0
    u = ONESB.view(np.uint32)
    keep = np.uint32(0xFFFFF000)
    half_ = np.uint32(0x800)
    lsb = (u >> np.uint32(12)) & np.uint32(1)
    ONESB = ((u + (half_ - np.uint32(1)) + lsb) & keep).view(np.float32)

    xTb = xT.astype(ml_dtypes.bfloat16)
    return {"XTB": xTb, "WQ": WQc, "WK": WKc, "WVA": WVA, "WP": WPc,
            "BQK": BQK, "BVA": BVA, "BP": BP.astype(np.float32),
            "QKN": QKN, "BLK2": BLK2, "SEL4": SEL4, "EPSV": EPSV,
            "ONESB": ONESB}


def kernel(x, W_qkv, b_qkv, W_proj, b_proj, qn_w, kn_w):
    from concourse.bass_utils import run_bass_kernel_spmd

    if "nc" not in _CACHE:
        _CACHE["nc"] = _build_nc()
    nc = _CACHE["nc"]

    args = (np.asarray(x, np.float32), np.asarray(W_qkv, np.float32),
            np.asarray(b_qkv, np.float32), np.asarray(W_proj, np.float32),
            np.asarray(b_proj, np.float32), np.asarray(qn_w, np.float32),
            np.asarray(kn_w, np.float32))
    in_maps = [_core_inputs(c, *args) for c in range(NCORES)]

    trace = os.environ.get("BASS_KERNEL_TRACE", "0") == "1"
    res = run_bass_kernel_spmd(nc, in_maps, core_ids=list(range(NCORES)),
                               trace=trace)
    LAST_RESULT[0] = res

    y = np.stack([res.results[2 * b]["Y"] + res.results[2 * b + 1]["Y"]
                  for b in range(B)])
    return y.astype(np.float32)


# revision 42
# speedup vs baseline: 1.0427x; 1.0427x over previous
"""Trainium2 Bass kernel for nn_Attention (B=4, N=2048, C=1024, H=16, D=64).

Sharding: 8 cores; core c handles batch b=c//2 and heads [8*(c%2), 8*(c%2)+8).
Each core computes qkv projection for its 512 channels, RMSNorm(q/k),
attention over its 8 heads, and a partial output projection (contraction over
its 512 channels). Host sums the two partial proj outputs per batch.

Schedule: the ACT-engine exp stream (256 x [128,1024] activations, ~290us)
is the critical path. All non-score PE work (qkv, V, stats, proj, norm) is
chopped into ~0.5-1.7us "fill quanta" drained by a leaky-bucket budget
between score pairs, so the in-order PE queue never starves the exp stream.
attn@V matmuls lag the sc/exp stream by >=2 j-tiles (po queue) so the PE
never head-of-line blocks on the exp it just issued.

PSUM: sc pool 2x[128,1024] (4 banks) + fill pool 2x[128,<=512] (2 banks)
+ po pool 2x[65,512] (2 banks) = 8 banks.

Stats: per head-pair, variances for (q,k)x(2 heads)x2048 tokens pack into
two [128,512] PSUM tiles (8 live rows each, 32-aligned for tile_position),
so Ln+Exp cost ~2.4us/hp on ACT and fit 1-bank fill tiles. ln/exp share one
pinned activation table (no table swaps).
"""

import os
import numpy as np
import ml_dtypes

B, N, C, H, D = 4, 2048, 1024, 16, 64
NCORES = 8
HPC = 8           # heads per core
CH = HPC * D      # 512 channels per core
VSEG = 2 * D + 4  # 132 cols per pair in v_aug: [64 v | 1 | 1][64 v | 1 | 1]
VW = 4 * VSEG     # 528
EPS = 1e-6

_CACHE = {}
LAST_RESULT = [None]


def _build_nc():
    import concourse.tile as tile
    import concourse.mybir as mybir
    from concourse import bacc

    F32 = mybir.dt.float32
    F32R = mybir.dt.float32r
    BF16 = mybir.dt.bfloat16
    AF = mybir.ActivationFunctionType

    class PinnedBacc(bacc.Bacc):
        """Route ln/exp to the shared natural_log_exp table so the ACT
        engine never swaps activation tables mid-kernel (each swap is
        ~1.3us and stalls the exp stream)."""

        def insert_act_table_loads(self):
            import bass_rust as _bass_rust
            from concourse.hw_specs import get_activation_tables
            has_activation = any(
                isinstance(i, mybir.InstActivation)
                for b in self.main_func.blocks
                for i in b.instructions
            )
            if not has_activation:
                return
            tables = []
            for name, fns in get_activation_tables(self.m.arch).items():
                if name != "natural_log_exp_and_others":
                    fns = {f for f in fns
                           if f.name.lower() not in ("exp", "ln")}
                tables.append((name, fns))
            _bass_rust.insert_act_table_loads(self, tables)

    nc = PinnedBacc("TRN2", target_bir_lowering=False, debug=False,
                    num_devices=NCORES)

    XTB = nc.dram_tensor("XTB", [C, N], BF16, kind="ExternalInput")
    WQ = nc.dram_tensor("WQ", [C, CH], BF16, kind="ExternalInput")
    WK = nc.dram_tensor("WK", [C, CH], BF16, kind="ExternalInput")
    WVA = nc.dram_tensor("WVA", [C, VW], BF16, kind="ExternalInput")
    WP = nc.dram_tensor("WP", [CH, C], BF16, kind="ExternalInput")
    BQK = nc.dram_tensor("BQK", [128, 8], F32, kind="ExternalInput")
    BVA = nc.dram_tensor("BVA", [128, VW], F32, kind="ExternalInput")
    BP = nc.dram_tensor("BP", [128, C], F32, kind="ExternalInput")
    QKN = nc.dram_tensor("QKN", [128, 2], F32, kind="ExternalInput")
    BLK2 = nc.dram_tensor("BLK2", [128, 64], BF16, kind="ExternalInput")
    SEL4 = nc.dram_tensor("SEL4", [128, 512], BF16, kind="ExternalInput")
    EPSV = nc.dram_tensor("EPSV", [128, 1], F32, kind="ExternalInput")
    ONESB = nc.dram_tensor("ONESB", [97, 64], F32R, kind="ExternalInput")
    Y = nc.dram_tensor("Y", [N, C], F32, kind="ExternalOutput")

    NT = N // 128          # 16 token tiles
    KT = C // 128          # 8 contraction tiles
    NCHUNK = N // 512      # 4 i-chunks of 512 tokens

    with tile.TileContext(nc) as tc:
        from contextlib import ExitStack
        from collections import deque
        with ExitStack() as ctx:
            const_p = ctx.enter_context(tc.tile_pool(name="const", bufs=1))
            xtb_p = ctx.enter_context(tc.tile_pool(name="xtb", bufs=8))
            outT_p = ctx.enter_context(tc.tile_pool(name="outT", bufs=4))
            wp_p = ctx.enter_context(tc.tile_pool(name="wp", bufs=4))
            y_p = ctx.enter_context(tc.tile_pool(name="y", bufs=2))

            sc_ps = ctx.enter_context(
                tc.tile_pool(name="scps", bufs=2, space="PSUM"))
            fill_ps = ctx.enter_context(
                tc.tile_pool(name="fillps", bufs=2, space="PSUM"))
            po_ps = ctx.enter_context(
                tc.tile_pool(name="po", bufs=2, space="PSUM"))

            # small constants
            bqk_sb = const_p.tile([128, 8], F32, tag="bqk")
            nc.sync.dma_start(bqk_sb[:], BQK.ap()[:, :])
            qkn_sb = const_p.tile([128, 2], F32, tag="qkn")
            nc.sync.dma_start(qkn_sb[:], QKN.ap()[:, :])
            blk2_sb = const_p.tile([128, 64], BF16, tag="blk2")
            nc.sync.dma_start(blk2_sb[:], BLK2.ap()[:, :])
            sel4_sb = const_p.tile([128, 512], BF16, tag="sel4")
            nc.sync.dma_start(sel4_sb[:], SEL4.ap()[:, :])
            epsv_sb = const_p.tile([128, 1], F32, tag="epsv")
            nc.sync.dma_start(epsv_sb[:], EPSV.ap()[:, :])
            onesb_sb = const_p.tile([97, 64], F32R, tag="onesb")
            nc.sync.dma_start(onesb_sb[:], ONESB.ap()[:, :])

            xtb_tiles = []
            for kt in range(KT):
                t = xtb_p.tile([128, N], BF16, tag="xtb", name=f"xtb{kt}")
                for h in range(2):
                    nc.sync.dma_start(
                        t[:, h * 1024:(h + 1) * 1024],
                        XTB.ap()[kt * 128:(kt + 1) * 128,
                                 h * 1024:(h + 1) * 1024])
                xtb_tiles.append(t)

            # ---------------- V tiles (bf16) -------------------------------
            v_ctx = tc.tile_pool(name="v", bufs=16)
            v_p = v_ctx.__enter__()
            v_tiles = []
            for nt in range(NT):
                v_tiles.append(
                    v_p.tile([128, VW], BF16, tag="v", name=f"vt{nt}"))

            pair_ctx = ExitStack()
            w_p = pair_ctx.enter_context(tc.tile_pool(name="w", bufs=1))
            qtb_p = pair_ctx.enter_context(tc.tile_pool(name="qtb", bufs=1))
            sq_p = pair_ctx.enter_context(tc.tile_pool(name="sq", bufs=4))
            qtn_p = pair_ctx.enter_context(tc.tile_pool(name="qtn", bufs=2))
            stat_p = pair_ctx.enter_context(tc.tile_pool(name="stat", bufs=4))
            rcp_p = pair_ctx.enter_context(tc.tile_pool(name="rcp", bufs=1))
            ex_p = pair_ctx.enter_context(tc.tile_pool(name="ex", bufs=6))

            wv_ctx = tc.tile_pool(name="wv", bufs=1)
            wv_p = wv_ctx.__enter__()

            state = {}

            # ============ fill-quantum / po scheduling machinery ==========
            fillq = deque()       # (cost_ns, emit_fn)
            poq = deque()         # (hp, ic, jt, emit_fn)
            sched = {"v_done": 0, "jt_global": 0, "budget": 0.0,
                     "qtn_q": [0] * 4, "ktn_q": [0] * 4}
            JT_BUDGET = 485.0     # fill ns per emitted jt (ACT slack)

            def push_fill(cost, fn):
                fillq.append((cost, fn))

            def pop_fill_budget():
                while fillq and fillq[0][0] <= sched["budget"]:
                    cost, fn = fillq.popleft()
                    sched["budget"] -= cost
                    fn()

            def flush_fill(n=None):
                cnt = 0
                while fillq and (n is None or cnt < n):
                    cost, fn = fillq.popleft()
                    fn()
                    cnt += 1
                sched["budget"] = min(sched["budget"], 0.0)

            # ---------------- emit helpers --------------------------------
            def emit_v_nt(nt, vh):
                # one quantum: half of v-tile nt (8 matmuls of 264 cols)
                cs = vh * (VW // 2)
                ps = fill_ps.tile([128, VW // 2], F32, tag="fill")
                for kt in range(KT):
                    nc.tensor.matmul(
                        ps[:],
                        xtb_tiles[kt][:, nt * 128:(nt + 1) * 128],
                        wv_sb[:, kt * VW + cs:kt * VW + cs + VW // 2],
                        start=(kt == 0), stop=(kt == KT - 1))
                nc.vector.tensor_add(
                    v_tiles[nt][:, cs:cs + VW // 2], ps[:],
                    bva_sb[:, cs:cs + VW // 2])
                if vh == 1:
                    sched["v_done"] = nt + 1

            def emit_w_loads(hp):
                wq_sb = w_p.tile([128, C], BF16, tag="wq")
                wk_sb = w_p.tile([128, C], BF16, tag="wk")
                for kt in range(KT):
                    nc.sync.dma_start(
                        wq_sb[:, kt * 128:(kt + 1) * 128],
                        WQ.ap()[kt * 128:(kt + 1) * 128,
                                hp * 128:(hp + 1) * 128])
                    nc.sync.dma_start(
                        wk_sb[:, kt * 128:(kt + 1) * 128],
                        WK.ap()[kt * 128:(kt + 1) * 128,
                                hp * 128:(hp + 1) * 128])
                st = state.setdefault(hp, {})
                st["wq"], st["wk"] = wq_sb, wk_sb
                st["qT_b"] = qtb_p.tile([128, N], BF16, tag="qtb",
                                        name=f"qTb{hp}")
                st["kT_b"] = qtb_p.tile([128, N], BF16, tag="ktb",
                                        name=f"kTb{hp}")

            def emit_qkv_pass(hp, is_k, qq):
                # one quantum: 8-ktile accumulation into [128,512] + bias
                st = state[hp]
                wsb = st["wk"] if is_k else st["wq"]
                dst = st["kT_b"] if is_k else st["qT_b"]
                bcol = (4 + hp) if is_k else hp
                ssl = slice(qq * 512, (qq + 1) * 512)
                ps = fill_ps.tile([128, 512], F32, tag="fill")
                for kt in range(KT):
                    nc.tensor.matmul(
                        ps[:], wsb[:, kt * 128:(kt + 1) * 128],
                        xtb_tiles[kt][:, ssl],
                        start=(kt == 0), stop=(kt == KT - 1))
                nc.vector.tensor_scalar(
                    dst[:, ssl], ps[:], bqk_sb[:, bcol:bcol + 1],
                    None, op0=mybir.AluOpType.add)

            def emit_stats_var_T(hp, T):
                # one quantum: variance tile T (token quarters 2T, 2T+1).
                # rows: q@64s..+1, k@64s+32..+33 for s in {0,1}; one Ln +
                # one Exp of [128,512] each.
                st = state[hp]
                qT_b, kT_b = st["qT_b"], st["kT_b"]
                vps = fill_ps.tile([128, 512], F32, tag="fill",
                                   name=f"vps{hp}_{T}")
                for s in range(2):
                    qq = 2 * T + s
                    ssl = slice(qq * 512, (qq + 1) * 512)
                    for qk, src_t in ((0, qT_b), (1, kT_b)):
                        sq = sq_p.tile([128, 512], BF16, tag="sq")
                        nc.gpsimd.tensor_mul(sq[:], src_t[:, ssl],
                                             src_t[:, ssl])
                        r = 64 * s + 32 * qk
                        bcols = slice(32, 64) if qk else slice(0, 32)
                        nc.tensor.matmul(
                            vps[r:r + 32, :], blk2_sb[:, bcols], sq[:],
                            start=True, stop=True, tile_position=(0, r))
                lg = stat_p.tile([128, 512], F32, tag="lg",
                                 name=f"lg{hp}_{T}")
                nc.scalar.activation(lg[:], vps[:], AF.Ln, bias=epsv_sb[:])
                rs = stat_p.tile([128, 512], BF16, tag="rs",
                                 name=f"rs{hp}_{T}")
                nc.scalar.activation(rs[:], lg[:], AF.Exp, scale=-0.5)
                st.setdefault("rs", {})[T] = rs

            def emit_stats_bcast_q(hp, is_k, qq):
                # one quantum: rsqrt broadcast + normalize for one
                # (q/k, token-quarter): [128,512]
                st = state[hp]
                src_t = st["kT_b"] if is_k else st["qT_b"]
                if "qTn" not in st:
                    st["qTn"] = qtn_p.tile([128, N], BF16, tag="qtn",
                                           name=f"qTn{hp}")
                    st["kTn"] = qtn_p.tile([128, N], BF16, tag="ktn",
                                           name=f"kTn{hp}")
                dstn = st["kTn"] if is_k else st["qTn"]
                rs = st["rs"][qq // 2]
                vi = 2 * (qq % 2) + int(is_k)
                ssl = slice(qq * 512, (qq + 1) * 512)
                bc = fill_ps.tile([128, 512], F32, tag="fill", name="bc")
                nc.tensor.matmul(
                    bc[:], sel4_sb[:, vi * 128:(vi + 1) * 128], rs[:],
                    start=True, stop=True)
                wcol = 1 if is_k else 0
                nc.vector.scalar_tensor_tensor(
                    dstn[:, ssl], src_t[:, ssl],
                    qkn_sb[:, wcol:wcol + 1], bc[:],
                    op0=mybir.AluOpType.mult,
                    op1=mybir.AluOpType.mult)
                key = "ktn_q" if is_k else "qtn_q"
                sched[key][hp] = max(sched[key][hp], qq + 1)

            outT_tiles = []

            def emit_attention_start(hp):
                st = state[hp]
                outT = outT_p.tile([128, N], BF16, tag="outT",
                                   name=f"outT{hp}")
                outT_tiles.append(outT)
                st["outT"] = outT
                # hp<3: dense [8,512] pack (rows 2*ic+hh) -> one batched
                # reciprocal. hp3: rows 32*ic+hh so per-ic reciprocal
                # reads start 32-aligned (engine-op requirement).
                st["den"] = rcp_p.tile([98, 512] if hp == 3 else [8, 512],
                                       BF16, tag="den_pack",
                                       bufs=2, name=f"den{hp}")
                st["po_sbs"] = {}

            def emit_po_jt(hp, ic, jt, poA, poB):
                st = state[hp]
                ex = st["exs"].pop((ic, jt))
                vbase = hp * VSEG
                nc.tensor.matmul(
                    poA[:], v_tiles[jt][:, vbase:vbase + 65],
                    ex[:, 0:512], start=(jt == 0), stop=(jt == NT - 1))
                nc.tensor.matmul(
                    poB[:],
                    v_tiles[jt][:, vbase + VSEG // 2:
                                vbase + VSEG // 2 + 65],
                    ex[:, 512:1024], start=(jt == 0), stop=(jt == NT - 1))
                if jt == NT - 1:
                    emit_den_extract(hp, ic, poA, poB)

            def emit_den_extract(hp, ic, poA, poB):
                st = state[hp]
                for hh, (po, rowoff) in enumerate(((poA, 0), (poB, 64))):
                    idx = ic * 2 + hh
                    po_sb = rcp_p.tile([65, 512], BF16, tag="po_sb",
                                       name=f"po_sb{hp}_{idx}", bufs=8)
                    nc.vector.tensor_copy(po_sb[:], po[:, :])
                    drow = (32 * ic + hh) if hp == 3 else (2 * ic + hh)
                    nc.sync.dma_start(
                        st["den"][drow:drow + 1, :], po_sb[64:65, :])
                    st["po_sbs"][idx] = (po_sb, rowoff, ic)
                if hp == 3:
                    push_hp3_post(ic)
                elif ic == 3:
                    push_norm_quanta(hp)

            def _rcp_al_dma(hp, ic, rcp_pack, row0):
                st = state[hp]
                rcp_al = rcp_p.tile([33, 512], F32, tag="rcp_al",
                                    bufs=4, name=f"rcpa{hp}_{ic}")
                st[f"rcp_al{ic}"] = rcp_al
                for i in range(2):
                    nc.sync.dma_start(rcp_al[32 * i:32 * i + 1, :],
                                      rcp_pack[row0 + i:row0 + i + 1, :])

            def emit_norm_rcp(hp):
                st = state[hp]
                rcp_pack = rcp_p.tile([8, 512], F32, tag="rcp_pack",
                                      bufs=2, name=f"rcpp{hp}")
                with nc.allow_low_precision(
                        reason="f32 reciprocal of softmax denominators"):
                    nc.vector.reciprocal(rcp_pack[:], st["den"][:, :])
                for ic in range(NCHUNK):
                    _rcp_al_dma(hp, ic, rcp_pack, 2 * ic)

            def emit_norm_rcp_ic(hp, ic):
                st = state[hp]
                rcp_pack = rcp_p.tile([2, 512], F32, tag="rcp_pack",
                                      bufs=2, name=f"rcpp{hp}_{ic}")
                with nc.allow_low_precision(
                        reason="f32 reciprocal of softmax denominators"):
                    nc.vector.reciprocal(
                        rcp_pack[:], st["den"][32 * ic:32 * ic + 2, :])
                _rcp_al_dma(hp, ic, rcp_pack, 0)

            def emit_norm_apply(hp, ic):
                # broadcast rcp rows via ONES matmul and scale po -> outT
                st = state[hp]
                outT = st["outT"]
                isl = slice(ic * 512, (ic + 1) * 512)
                rcp_al = st[f"rcp_al{ic}"]
                for hh in range(2):
                    po_sb, rowoff, _ = st["po_sbs"][2 * ic + hh]
                    r = 32 * hh
                    rb_ps = fill_ps.tile([64, 512], F32, tag="fill",
                                         name="rb_ps")
                    nc.tensor.matmul(
                        rb_ps[:], onesb_sb[r:r + 1, :],
                        rcp_al[r:r + 1, :].bitcast(F32R),
                        start=True, stop=True, tile_position=(r, 0))
                    nc.vector.tensor_mul(
                        outT[rowoff:rowoff + 64, isl], po_sb[0:64, :],
                        rb_ps[:])

            y_sbs = {}

            def emit_proj_sub(nt, sub):
                # one quantum: half of proj output tile nt
                ps = fill_ps.tile([128, 512], F32, tag="fill")
                for kt in range(4):
                    nc.tensor.matmul(
                        ps[:], outT_tiles[kt][:, nt * 128:(nt + 1) * 128],
                        wp_tiles[kt][:, sub * 512:(sub + 1) * 512],
                        start=(kt == 0), stop=(kt == 3))
                if sub == 0:
                    y_sbs[nt] = y_p.tile([128, C], F32, tag="y",
                                         name=f"y{nt}")
                y_sb = y_sbs[nt]
                ssl = slice(sub * 512, (sub + 1) * 512)
                nc.vector.tensor_add(y_sb[:, ssl], ps[:], bp_sb[:, ssl])
                if sub == 1:
                    nc.sync.dma_start(Y.ap()[nt * 128:(nt + 1) * 128, :],
                                      y_sb[:])

            def push_norm_quanta(hp):
                # after hp's last den row lands: batched rcp, then one
                # apply-quantum per i-chunk
                push_fill(900, lambda hp=hp: emit_norm_rcp(hp))
                for ic in range(NCHUNK):
                    push_fill(500, lambda hp=hp, ic=ic:
                              emit_norm_apply(hp, ic))

            def push_hp3_post(ic):
                # hp3 per-ic: rcp + apply + the 4 proj tiles it unlocks
                push_fill(600, lambda ic=ic: emit_norm_rcp_ic(3, ic))
                push_fill(500, lambda ic=ic: emit_norm_apply(3, ic))
                for nt in range(4 * ic, 4 * ic + 4):
                    for sub in range(2):
                        push_fill(900, lambda nt=nt, sub=sub:
                                  emit_proj_sub(nt, sub))

            # ---------------- attention jt stream --------------------------
            def emit_sc_jt(hp, ic, jt):
                # correctness guard: kTn quarter jt//4 and qTn quarter ic
                # must be emitted before this score pair reads them
                while (sched["ktn_q"][hp] <= jt // 4
                       or sched["qtn_q"][hp] <= ic):
                    assert fillq, "sc guard: missing bcast quanta"
                    flush_fill(n=1)
                st = state[hp]
                qTn, kTn = st["qTn"], st["kTn"]
                isl = slice(ic * 512, (ic + 1) * 512)
                jsl = slice(jt * 128, (jt + 1) * 128)
                sc = sc_ps.tile([128, 1024], F32, tag="sc")
                nc.tensor.matmul(
                    sc[:, 0:512], kTn[0:64, jsl], qTn[0:64, isl],
                    start=True, stop=True, tile_position=(0, 0))
                nc.tensor.matmul(
                    sc[:, 512:1024], kTn[64:128, jsl], qTn[64:128, isl],
                    start=True, stop=True, tile_position=(64, 0))
                ex = ex_p.tile([128, 1024], BF16, tag="ex")
                nc.scalar.activation(ex[:], sc[:], AF.Exp)
                st.setdefault("exs", {})[(ic, jt)] = ex

            def pop_po(max_n):
                n = 0
                while poq and n < max_n:
                    hp, ic, jt, born = poq[0]
                    if sched["v_done"] <= jt:
                        break
                    if sched["jt_global"] - born < 2:
                        break
                    poq.popleft()
                    po_pair = po_tiles[(hp, ic)]
                    emit_po_jt(hp, ic, jt, po_pair[0], po_pair[1])
                    n += 1

            po_tiles = {}

            # ================= emission =====================================
            # DMA priority: wq/wk hp0 (gates first qkv), wv, then the
            # late-needed BVA/BP/WP constants.
            emit_w_loads(0)
            wv_sb = wv_p.tile([128, KT * VW], BF16, tag="wv")
            for kt in range(KT):
                nc.sync.dma_start(wv_sb[:, kt * VW:(kt + 1) * VW],
                                  WVA.ap()[kt * 128:(kt + 1) * 128, :])
            bva_sb = const_p.tile([128, VW], F32, tag="bva")
            nc.sync.dma_start(bva_sb[:], BVA.ap()[:, :])
            bp_sb = const_p.tile([128, C], F32, tag="bp")
            nc.sync.dma_start(bp_sb[:], BP.ap()[:, :])
            wp_tiles = []
            for kt in range(4):
                t = wp_p.tile([128, C], BF16, tag="wp")
                nc.sync.dma_start(t[:], WP.ap()[kt * 128:(kt + 1) * 128, :])
                wp_tiles.append(t)

            # pre-loop: qkv(0) + stats(0) + bcast(0) direct; stats T0
            # needs only token quarters 0-1, so it overlaps the q2/q3
            # qkv passes (shortens the first-exp critical path)
            for qq in range(2):
                emit_qkv_pass(0, False, qq)
                emit_qkv_pass(0, True, qq)
            emit_stats_var_T(0, 0)
            for qq in range(2, 4):
                emit_qkv_pass(0, False, qq)
                emit_qkv_pass(0, True, qq)
            emit_stats_var_T(0, 1)
            emit_stats_bcast_q(0, False, 0)
            emit_stats_bcast_q(0, True, 0)
            emit_stats_bcast_q(0, False, 1)
            emit_stats_bcast_q(0, True, 1)
            for qq in range(2, 4):
                emit_stats_bcast_q(0, False, qq)
                emit_stats_bcast_q(0, True, qq)

            # hp0 fill: V (16 tiles, 2 quanta each) interleaved with
            # qkv(1); then stats(1) + bcast(1)
            emit_w_loads(1)
            qkv1 = [(False, qq) for qq in range(4)] + \
                   [(True, qq) for qq in range(4)]
            for nt in range(NT):
                push_fill(880, lambda nt=nt: emit_v_nt(nt, 0))
                push_fill(880, lambda nt=nt: emit_v_nt(nt, 1))
                if nt % 2 == 1 and qkv1:
                    is_k, qq = qkv1.pop(0)
                    push_fill(1700, lambda is_k=is_k, qq=qq:
                              emit_qkv_pass(1, is_k, qq))
            for T in range(2):
                push_fill(900, lambda T=T: emit_stats_var_T(1, T))
            for qq in range(4):
                for is_k in (False, True):
                    push_fill(250, lambda is_k=is_k, qq=qq:
                              emit_stats_bcast_q(1, is_k, qq))

            for hp in range(4):
                if hp >= 1:
                    # fill with a deadline this hp (qTn/kTn broadcast)
                    # must be fully emitted before this hp's first scores
                    flush_fill()
                emit_attention_start(hp)
                if hp == 1:
                    wv_ctx.__exit__(None, None, None)
                if 1 <= hp <= 2:
                    nhp = hp + 1
                    emit_w_loads(nhp)
                    for is_k in (False, True):
                        for qq in range(4):
                            push_fill(1700, lambda n=nhp, is_k=is_k, qq=qq:
                                      emit_qkv_pass(n, is_k, qq))
                    for T in range(2):
                        push_fill(900, lambda n=nhp, T=T:
                                  emit_stats_var_T(n, T))
                    for qq in range(4):
                        for is_k in (False, True):
                            push_fill(250, lambda n=nhp, is_k=is_k, qq=qq:
                                      emit_stats_bcast_q(n, is_k, qq))
                for ic in range(NCHUNK):
                    po_tiles[(hp, ic)] = (
                        po_ps.tile([65, 512], F32, tag="po",
                                   name=f"poA{hp}_{ic}"),
                        po_ps.tile([65, 512], F32, tag="po",
                                   name=f"poB{hp}_{ic}"))
                    for jt in range(NT):
                        emit_sc_jt(hp, ic, jt)
                        poq.append((hp, ic, jt, sched["jt_global"]))
                        sched["jt_global"] += 1
                        # hp0 is PE-bound (V production): drain fill faster
                        jb = 750.0 if hp == 0 else JT_BUDGET
                        sched["budget"] = min(
                            sched["budget"] + jb, 4 * jb)
                        pop_po(3 if len(poq) > 8 else 2)
                        pop_fill_budget()

            # tail: drain remaining po + fill
            while poq:
                hp, ic, jt, _ = poq.popleft()
                po_pair = po_tiles[(hp, ic)]
                emit_po_jt(hp, ic, jt, po_pair[0], po_pair[1])
                flush_fill(n=1)
            flush_fill()

            pair_ctx.close()
            v_ctx.__exit__(None, None, None)

    nc.compile()
    return nc


def _core_inputs(c, x, W_qkv, b_qkv, W_proj, b_proj, qn_w, kn_w):
    b, half = c // 2, c % 2
    hbase = HPC * half
    co = hbase * D                      # channel offset of this core's heads

    xT = np.ascontiguousarray(x[b].T, dtype=np.float32)
    WQc = W_qkv[:, co:co + CH].astype(ml_dtypes.bfloat16)
    WKc = W_qkv[:, C + co:C + co + CH].astype(ml_dtypes.bfloat16)
    WVc = W_qkv[:, 2 * C + co:2 * C + co + CH]
    WVA = np.zeros((C, VW), dtype=np.float32)
    BVA1 = np.zeros((VW,), dtype=np.float32)
    bv = b_qkv[2 * C + co:2 * C + co + CH]
    for hp in range(4):
        for hh in range(2):
            s = hp * VSEG + hh * (VSEG // 2)
            WVA[:, s:s + D] = WVc[:, (2 * hp + hh) * D:(2 * hp + hh + 1) * D]
            BVA1[s:s + D] = bv[(2 * hp + hh) * D:(2 * hp + hh + 1) * D]
            BVA1[s + D] = 1.0  # ones column for softmax denominators
    WVA = WVA.astype(ml_dtypes.bfloat16)
    BVA = np.broadcast_to(BVA1, (128, VW)).copy()

    BQK = np.zeros((128, 8), dtype=np.float32)
    for hp in range(4):
        BQK[:, hp] = b_qkv[co + hp * 128:co + (hp + 1) * 128]
        BQK[:, 4 + hp] = b_qkv[C + co + hp * 128:C + co + (hp + 1) * 128]

    WPc = W_proj[co:co + CH, :].astype(ml_dtypes.bfloat16)
    BP = (np.broadcast_to(b_proj, (128, C)).copy() if half == 0
          else np.zeros((128, C), dtype=np.float32))
    QKN = np.stack([np.tile(qn_w, 2), np.tile(kn_w, 2)],
                   axis=1).astype(np.float32)

    # variance matmul weights: col 0/1 q h0/h1 (1/D), col 32/33 k h0/h1
    # (1.0; the 1/D and the 1/sqrt(D) score scale fold into the rsqrt)
    BLK2 = np.zeros((128, 64), dtype=np.float32)
    BLK2[0:64, 0] = 1.0 / D
    BLK2[64:128, 1] = 1.0 / D
    BLK2[0:64, 32] = 1.0
    BLK2[64:128, 33] = 1.0
    BLK2 = BLK2.astype(ml_dtypes.bfloat16)

    # rsqrt broadcast selectors: variant vi reads rs rows 32*vi (h0) and
    # 32*vi+1 (h1) onto channels 0-63 / 64-127
    SEL4 = np.zeros((128, 512), dtype=np.float32)
    for vi in range(4):
        SEL4[32 * vi, vi * 128:vi * 128 + 64] = 1.0
        SEL4[32 * vi + 1, vi * 128 + 64:vi * 128 + 128] = 1.0
    SEL4 = SEL4.astype(ml_dtypes.bfloat16)

    # packed variance tiles: rows 0-31 q (bias EPS), 32-63 k (bias EPS*D),
    # 64-95 q, 96-127 k
    EPSV = np.full((128, 1), EPS, dtype=np.float32)
    EPSV[32:64] = EPS * D
    EPSV[96:128] = EPS * D

    ONESB = np.zeros((97, 64), dtype=np.float32)
    for r in (0, 32, 64, 96):
        ONESB[r, :] = 1.0
    u = ONESB.view(np.uint32)
    keep = np.uint32(0xFFFFF000)
    half_ = np.uint32(0x800)
    lsb = (u >> np.uint32(12)) & np.uint32(1)
    ONESB = ((u + (half_ - np.uint32(1)) + lsb) & keep).view(np.float32)

    xTb = xT.astype(ml_dtypes.bfloat16)
    return {"XTB": xTb, "WQ": WQc, "WK": WKc, "WVA": WVA, "WP": WPc,
            "BQK": BQK, "BVA": BVA, "BP": BP.astype(np.float32),
            "QKN": QKN, "BLK2": BLK2, "SEL4": SEL4, "EPSV": EPSV,
            "ONESB": ONESB}


def kernel(x, W_qkv, b_qkv, W_proj, b_proj, qn_w, kn_w):
    from concourse.bass_utils import run_bass_kernel_spmd

    if "nc" not in _CACHE:
        _CACHE["nc"] = _build_nc()
    nc = _CACHE["nc"]

    args = (np.asarray(x, np.float32), np.asarray(W_qkv, np.float32),
            np.asarray(b_qkv, np.float32), np.asarray(W_proj, np.float32),
            np.asarray(b_proj, np.float32), np.asarray(qn_w, np.float32),
            np.asarray(kn_w, np.float32))
    in_maps = [_core_inputs(c, *args) for c in range(NCORES)]

    trace = os.environ.get("BASS_KERNEL_TRACE", "0") == "1"
    res = run_bass_kernel_spmd(nc, in_maps, core_ids=list(range(NCORES)),
                               trace=trace)
    LAST_RESULT[0] = res

    y = np.stack([res.results[2 * b]["Y"] + res.results[2 * b + 1]["Y"]
                  for b in range(B)])
    return y.astype(np.float32)
